# revision 1
# baseline (speedup 1.0000x reference)
"""Causal self-attention (B=128, T=512, C=512, H=16) on 8 Trainium2 NeuronCores.

Sharding: data-parallel over batch — each core computes 16 of the 128
batch elements end-to-end; weights are replicated. No collectives.

Per-core kernel (Bass/Tile; matmul operands in bf16, fp32 accumulation):
  - x is transposed on-chip via the PE (identity matmul) so C sits on
    partitions.
  - qT/kT [f, t] = W_attn.T @ x.T with W_attn natural as the stationary
    operand; head h lands on partition rows 32*(h%4) of f-block h//4,
    giving 4-way PE row-group packing for the K=32 scores matmuls.
  - scores are computed transposed, S'[k, q] = kT.T @ qT, over causal
    column ranges padded to >=256 (fp32r needs N>=256 for 1 cyc/row and
    even N always).
  - E' = exp(S'/sqrt(D)) on ScalarE (scale fused); causal masking via
    gpsimd affine_select directly on the fp32r tiles.
  - yT[d, q] = [v | 1].T @ E' accumulated over k-blocks in PSUM; the ones
    column produces the softmax denominator in row 32 of the same PSUM.
  - normalization: DVE reciprocal of the denominator row, K=1 ones-matmul
    broadcast across partitions, DVE multiply; SBUF->SBUF DMA moves each
    head's 32 rows into the packed yT layout.
  - out = yT.T @ W_proj + b_proj streamed straight back to DRAM.

All fp32r-matmul operands are produced by compute ops writing
float32r-typed tiles (the walrus BIR verifier rejects unrounded fp32
producers feeding fp32r matmuls).
"""

import math
import sys

if "/opt/trn_rl_repo" not in sys.path:
    sys.path.insert(0, "/opt/trn_rl_repo")

import numpy as np

import concourse.tile as tile
from concourse import bacc, mybir
from concourse.bass_utils import run_bass_kernel_spmd
from concourse.masks import make_identity

F32 = mybir.dt.float32
F32R = mybir.dt.float32r
BF16 = mybir.dt.float16  # fp16: full matmul rate + FWL, 11-bit mantissa
AF = mybir.ActivationFunctionType
ALU = mybir.AluOpType

B, T, C, H, D = 128, 512, 512, 16, 32
P = 128
N_CORES = 8
BC = B // N_CORES       # 16 batches per core
NCBLK = C // P          # 4
NTBLK = T // P          # 4
HPG = 4                 # heads per 128-row f-block
NHG = H // HPG          # 4 head groups
SCALE = 1.0 / math.sqrt(D)
QR0 = [0, 128, 256, 384]  # q range start per k-block (exact causal)


def _build_kernel(tc, out, x, w_attn, b_attn, w_proj, b_proj, b_count=BC):
    nc = tc.nc
    with (
        tc.tile_pool(name="const", bufs=1) as const_pool,
        tc.tile_pool(name="xnat", bufs=2) as xnat_pool,
        tc.tile_pool(name="xt", bufs=2) as xt_pool,
        tc.tile_pool(name="qt", bufs=2) as qt_pool,
        tc.tile_pool(name="kt", bufs=2) as kt_pool,
        tc.tile_pool(name="vaug", bufs=2) as v_pool,
        tc.tile_pool(name="eprime", bufs=17) as e_pool,
        tc.tile_pool(name="ytn", bufs=2) as ytn_pool,
        tc.tile_pool(name="small", bufs=2) as small_pool,
        tc.tile_pool(name="obuf", bufs=3) as o_pool,
        tc.tile_pool(name="s_psum", bufs=5, space="PSUM") as s_psum,
        tc.tile_pool(name="y_psum", bufs=2, space="PSUM") as y_psum,
        tc.tile_pool(name="mm_psum", bufs=1, space="PSUM") as mm_psum,
    ):
        identity = const_pool.tile([P, P], F32, tag="identity")
        make_identity(nc, identity)
        onesf = const_pool.tile([P, P], F32, tag="onesf")
        nc.gpsimd.memset(onesf[:], 1.0)
        ones_r = const_pool.tile([P, P], BF16, tag="ones_r")
        nc.vector.tensor_copy(ones_r[:], onesf[:])

        # weights: DMA to fp32 staging, then round into fp32r tiles.
        # The staging pool is scoped so its SBUF is released before the
        # batch loop pools are sized.
        wqk = const_pool.tile([P, NCBLK, 2 * C], BF16, tag="wqk")
        wv = const_pool.tile([P, NCBLK, C], BF16, tag="wv")
        wp = const_pool.tile([P, NCBLK, C], BF16, tag="wp")
        battn = const_pool.tile([P, 8], F32, tag="battn")
        bv_row = const_pool.tile([1, C], BF16, tag="bv_row")
        bp_row = const_pool.tile([1, C], BF16, tag="bp_row")
        with tc.tile_pool(name="stage", bufs=2) as stage_pool:
            for co in range(NCBLK):
                st = stage_pool.tile([P, 2 * C], F32, tag="stage", name="st_qk")
                nc.sync.dma_start(
                    st[:],
                    w_attn[co * P : (co + 1) * P, : 2 * C],
                )
                nc.vector.tensor_copy(wqk[:, co, :], st[:])
                st2 = stage_pool.tile([P, 2 * C], F32, tag="stage", name="st_v")
                nc.sync.dma_start(
                    st2[:, :C], w_attn[co * P : (co + 1) * P, 2 * C :]
                )
                nc.vector.tensor_copy(wv[:, co, :], st2[:, :C])
                st3 = stage_pool.tile([P, 2 * C], F32, tag="stage", name="st_p")
                nc.sync.dma_start(st3[:, :C], w_proj[co * P : (co + 1) * P, :])
                nc.vector.tensor_copy(wp[:, co, :], st3[:, :C])

            nc.sync.dma_start(
                battn[:], b_attn[: 2 * C].rearrange("(g p) -> p g", p=P)
            )
            st4 = stage_pool.tile([P, 2 * C], F32, tag="stage", name="st_b")
            nc.sync.dma_start(
                st4[0:1, :C], b_attn[2 * C :].rearrange("(o f) -> o f", o=1)
            )
            nc.sync.dma_start(
                st4[0:1, C : 2 * C], b_proj.rearrange("(o f) -> o f", o=1)
            )
            nc.vector.tensor_copy(bv_row[:], st4[0:1, :C])
            nc.vector.tensor_copy(bp_row[:], st4[0:1, C : 2 * C])

        bv_bcast = const_pool.tile([P, C], F32, tag="bv_bcast")
        bp_bcast = const_pool.tile([P, C], F32, tag="bp_bcast")
        for row, bcast in ((bv_row, bv_bcast), (bp_row, bp_bcast)):
            ps = mm_psum.tile([P, C], F32, tag="mm", name="bps")
            nc.tensor.matmul(ps[:], ones_r[0:1, :], row[:], start=True, stop=True)
            nc.scalar.copy(bcast[:], ps[:])

        for b in range(b_count):
            # transpose x_b -> xT_b [c, t] (fp32 PE transpose, cast on evac)
            xt_b = xt_pool.tile([P, NCBLK, T], BF16, tag="xt")
            for tb in range(NTBLK):
                xn = xnat_pool.tile([P, C], F32, tag="xnat")
                nc.sync.dma_start(xn[:], x[b, tb * P : (tb + 1) * P, :])
                for cb in range(NCBLK):
                    tps = mm_psum.tile([P, P], F32, tag="mm", name="tps")
                    nc.tensor.transpose(
                        tps[:], xn[:, cb * P : (cb + 1) * P], identity[:]
                    )
                    nc.vector.tensor_copy(xt_b[:, cb, tb * P : (tb + 1) * P], tps[:])

            # QKV
            qt_b = qt_pool.tile([P, NHG, T], BF16, tag="qt")
            kt_b = kt_pool.tile([P, NHG, T], BF16, tag="kt")
            v_b = v_pool.tile([P, NTBLK, H, 2 * D], BF16, tag="vaug")
            nc.vector.tensor_copy(
                v_b[:, :, :, 0:D],
                onesf[:, 0:D]
                .rearrange("p (a h d) -> p a h d", a=1, h=1, d=D)
                .to_broadcast([P, NTBLK, H, D]),
            )

            for g in range(2 * NHG):
                ps = mm_psum.tile([P, T], F32, tag="mm", name="qkps")
                for co in range(NCBLK):
                    nc.tensor.matmul(
                        ps[:],
                        wqk[:, co, g * P : (g + 1) * P],
                        xt_b[:, co, :],
                        start=(co == 0),
                        stop=(co == NCBLK - 1),
                    )
                dst = qt_b[:, g, :] if g < NHG else kt_b[:, g - NHG, :]
                nc.vector.tensor_scalar_add(dst, ps[:], battn[:, g : g + 1])

            for tb in range(NTBLK):
                ps = mm_psum.tile([P, C], F32, tag="mm", name="vps")
                for co in range(NCBLK):
                    nc.tensor.matmul(
                        ps[:],
                        xt_b[:, co, tb * P : (tb + 1) * P],
                        wv[:, co, :],
                        start=(co == 0),
                        stop=(co == NCBLK - 1),
                    )
                nc.vector.tensor_tensor(
                    v_b[:, tb, :, D : 2 * D],
                    ps.rearrange("p (h d) -> p h d", h=H),
                    bv_bcast.rearrange("p (h d) -> p h d", h=H),
                    ALU.add,
                )

            # attention — software-pipelined: scores/exp for head pair N
            # overlap yT/normalize for pair N-1, keeping the PE stream dense
            # (HAM stays un-throttled).
            ytn_b = ytn_pool.tile([P, NCBLK, T], BF16, tag="ytn")

            def emit_yt(g, pair, e_tiles):
                for hp in pair:
                    h = g * HPG + hp
                    yps = y_psum.tile([2 * D, T], F32, tag="y", name="yps")
                    for i in range(NTBLK):
                        nc.tensor.matmul(
                            yps[:, QR0[i] : T],
                            v_b[:, i, h, :],
                            e_tiles[hp, i],
                            start=(i == 0),
                            stop=(i == NTBLK - 1),
                        )
                    rec = small_pool.tile([1, T], F32, tag="rec")
                    # ~5x faster than reciprocal(); 51 ULP is far below the
                    # fp32r matmul rounding already present. Custom DVE ops
                    # only read base partition 0, hence sums in psum row 0.
                    nc.vector.reciprocal_approx_fast(rec[0:1, :], yps[0:1, :])
                    bcast_sb = small_pool.tile([D, T], F32, tag="bcast")
                    nc.gpsimd.partition_broadcast(
                        bcast_sb[:], rec[0:1, :], channels=D
                    )
                    ytmp = small_pool.tile([D, T], BF16, tag="ytmp")
                    nc.vector.tensor_tensor(
                        ytmp[:], yps[D : 2 * D, :], bcast_sb[:], ALU.mult
                    )
                    nc.sync.dma_start(
                        ytn_b[32 * hp : 32 * hp + 32, g, :], ytmp[:]
                    )

            pending = None
            for g in range(NHG):
                for pair in ((0, 1), (2, 3)):
                    e_tiles = {}
                    for i in range(NTBLK):
                        q0 = QR0[i]
                        n = T - q0
                        for hp in pair:
                            sps_t = s_psum.tile([P, T], F32, tag="s", name="sps")
                            sps = sps_t[:, :n]
                            nc.tensor.matmul(
                                sps,
                                kt_b[32 * hp : 32 * hp + 32, g, i * P : (i + 1) * P],
                                qt_b[32 * hp : 32 * hp + 32, g, q0:T],
                                start=True,
                                stop=True,
                                tile_position=(32 * hp, 0),
                            )
                            ep_t = e_pool.tile([P, T], BF16, tag="e", name="ep")
                            ep = ep_t[:, :n]
                            nc.scalar.activation(ep, sps, AF.Exp, scale=SCALE)
                            # causal mask on the diagonal block (local cols
                            # [0, 128)): keep iff f - p >= 0.
                            nc.gpsimd.affine_select(
                                out=ep[:, 0:P],
                                in_=ep[:, 0:P],
                                compare_op=ALU.is_ge,
                                fill=0.0,
                                base=0,
                                channel_multiplier=-1,
                                pattern=[[1, P]],
                            )
                            e_tiles[hp, i] = ep
                    if pending is not None:
                        emit_yt(*pending)
                    pending = (g, pair, e_tiles)
            emit_yt(*pending)

            # output projection
            for tb in range(NTBLK):
                ps = mm_psum.tile([P, C], F32, tag="mm", name="ops")
                for cb in range(NCBLK):
                    nc.tensor.matmul(
                        ps[:],
                        ytn_b[:, cb, tb * P : (tb + 1) * P],
                        wp[:, cb, :],
                        start=(cb == 0),
                        stop=(cb == NCBLK - 1),
                    )
                ob = o_pool.tile([P, C], F32, tag="obuf")
                nc.vector.tensor_tensor(ob[:], ps[:], bp_bcast[:], ALU.add)
                nc.sync.dma_start(out[b, tb * P : (tb + 1) * P, :], ob[:])


_NC_CACHE = None


def build_nc(b_count: int = BC, num_devices: int = N_CORES):
    nc = bacc.Bacc(
        "TRN2", target_bir_lowering=False, debug=False, num_devices=num_devices
    )
    x = nc.dram_tensor("x", [b_count, T, C], F32, kind="ExternalInput").ap()
    w_attn = nc.dram_tensor("w_attn", [C, 3 * C], F32, kind="ExternalInput").ap()
    b_attn = nc.dram_tensor("b_attn", [3 * C], F32, kind="ExternalInput").ap()
    w_proj = nc.dram_tensor("w_proj", [C, C], F32, kind="ExternalInput").ap()
    b_proj = nc.dram_tensor("b_proj", [C], F32, kind="ExternalInput").ap()
    out = nc.dram_tensor("out", [b_count, T, C], F32, kind="ExternalOutput").ap()
    with tile.TileContext(nc) as tc:
        _build_kernel(tc, out, x, w_attn, b_attn, w_proj, b_proj, b_count)
    nc.compile()
    return nc


def _get_nc():
    global _NC_CACHE
    if _NC_CACHE is None:
        _NC_CACHE = build_nc(BC, N_CORES)
    return _NC_CACHE


def kernel(x, W_attn, b_attn, W_proj, b_proj):
    x = np.ascontiguousarray(np.asarray(x, dtype=np.float32))
    W_attn = np.ascontiguousarray(np.asarray(W_attn, dtype=np.float32))
    b_attn = np.ascontiguousarray(np.asarray(b_attn, dtype=np.float32))
    W_proj = np.ascontiguousarray(np.asarray(W_proj, dtype=np.float32))
    b_proj = np.ascontiguousarray(np.asarray(b_proj, dtype=np.float32))

    nc = _get_nc()
    in_maps = [
        {
            "x": x[c * BC : (c + 1) * BC],
            "w_attn": W_attn,
            "b_attn": b_attn,
            "w_proj": W_proj,
            "b_proj": b_proj,
        }
        for c in range(N_CORES)
    ]
    res = run_bass_kernel_spmd(nc, in_maps, core_ids=list(range(N_CORES)))
    return np.concatenate([res.results[c]["out"] for c in range(N_CORES)], axis=0)



# revision 22
# speedup vs baseline: 1.6899x; 1.6899x over previous
"""Causal self-attention (B=128, T=512, C=512, H=16) on 8 Trainium2 NeuronCores.

Sharding: data-parallel over batch — each core computes 16 of the 128
batch elements end-to-end; weights are replicated. No collectives.

Per-core kernel (Bass/Tile; matmul operands fp16, fp32 accumulation),
restructured from the v1 kernel around three findings from its trace:
  1. HAM throttling: the PE ran at K=4/8 (1.2 GHz) for 82% of the run
     because per-batch serialization (normalize tail -> proj -> next
     batch through one shared PSUM bank) left 200ns-5.4us PE gaps.
     The emission schedule now interleaves, at head-group granularity,
     batch b's attention with batch b+1's transpose/QKV and batch b-1's
     output projection so the PE stream stays dense.
  2. ScalarE exp paid a 352-cycle fixed cost on each of 64 small
     ACTIVATEs per batch. Scores for the 4 heads of a group now land in
     4 PSUM banks of one 4-bank tile and are exp'd by ONE ACTIVATE over
     a bank-strided AP (16 per batch).
  3. The normalize chain (1-partition reciprocal + gpsimd
     partition_broadcast per head, 440us combined) is gone: each y-PSUM
     bank col-tiles y of two heads with the ones-matmul denominators of
     the OTHER two heads placed at complementary partition rows, so a
     full-bank reciprocal_approx_fast of bank A lands 1/den exactly on
     the partitions where bank B holds y (and vice versa); three DVE
     multiplies then write normalized yT directly into the packed
     stationary layout for the projection (W_proj rows permuted at load
     to match).

Causal masking multiplies the diagonal 128-block of each exp'd tile by
a constant lower-triangle mask on GpSimd (one op per (group, k-block)).
"""

import math
import sys

if "/opt/trn_rl_repo" not in sys.path:
    sys.path.insert(0, "/opt/trn_rl_repo")

import numpy as np

import concourse.tile as tile
from concourse import bacc, mybir
from concourse.bass_utils import run_bass_kernel_spmd
from concourse.masks import make_identity

F32 = mybir.dt.float32
F16 = mybir.dt.float16  # fp16: full matmul rate + FWL, 11-bit mantissa
AF = mybir.ActivationFunctionType
ALU = mybir.AluOpType

B, T, C, H, D = 128, 512, 512, 16, 32
P = 128
N_CORES = 8
BC = B // N_CORES       # 16 batches per core
NCBLK = C // P          # 4
NTBLK = T // P          # 4
HPG = 4                 # heads per 128-row f-block
NHG = H // HPG          # 4 head groups
SCALE = 1.0 / math.sqrt(D)
# ytn row-group j of f-block g holds head 4g + JPERM[j] (see y-bank layout)
JPERM = [0, 1, 2, 3]


def _build_kernel(tc, out, x, w_attn, b_attn, w_proj, b_proj, b_count=BC):
    nc = tc.nc
    with (
        tc.tile_pool(name="const", bufs=1) as const_pool,
        tc.tile_pool(name="xn", bufs=2) as xn_pool,
        tc.tile_pool(name="xt", bufs=2) as xt_pool,
        tc.tile_pool(name="qt", bufs=2) as qt_pool,
        tc.tile_pool(name="kt", bufs=2) as kt_pool,
        tc.tile_pool(name="vnat", bufs=2) as v_pool,
        tc.tile_pool(name="eprime", bufs=2) as e_pool,
        tc.tile_pool(name="rec", bufs=4) as rec_pool,
        tc.tile_pool(name="ytn", bufs=2) as ytn_pool,
        tc.tile_pool(name="obuf", bufs=3) as o_pool,
        tc.tile_pool(name="s_psum", bufs=1, space="PSUM") as s_psum,
        tc.tile_pool(name="y_psum", bufs=2, space="PSUM") as y_psum,
        tc.tile_pool(name="mm_psum", bufs=2, space="PSUM") as mm_psum,
    ):
        identity = const_pool.tile([P, P], F32, tag="identity")
        make_identity(nc, identity)
        ones32 = const_pool.tile([P, D], F16, tag="ones32")
        nc.gpsimd.memset(ones32[:], 1.0)
        # lower-triangle keep mask for the causal diagonal block:
        # masktri[p, f] = 1.0 iff f >= p
        masktri = const_pool.tile([P, P], F16, tag="masktri")
        nc.gpsimd.memset(masktri[:], 1.0)
        nc.gpsimd.affine_select(
            out=masktri[:],
            in_=masktri[:],
            compare_op=ALU.is_ge,
            fill=0.0,
            base=0,
            channel_multiplier=-1,
            pattern=[[1, P]],
        )
        onesf = const_pool.tile([P, P], F32, tag="onesf")
        nc.gpsimd.memset(onesf[:], 1.0)
        ones_r = const_pool.tile([P, P], F16, tag="ones_r")
        nc.vector.tensor_copy(ones_r[:], onesf[:])

        wqk = const_pool.tile([P, NCBLK, 2 * C], F16, tag="wqk")
        wv = const_pool.tile([P, NCBLK, C], F16, tag="wv")
        wp = const_pool.tile([P, NCBLK, C], F16, tag="wp")
        battn = const_pool.tile([P, 8], F32, tag="battn")
        bv_row = const_pool.tile([1, C], F16, tag="bv_row")
        with tc.tile_pool(name="stage", bufs=2) as stage_pool:
            for co in range(NCBLK):
                st = stage_pool.tile([P, 2 * C], F32, tag="stage", name="st_qk")
                nc.sync.dma_start(st[:], w_attn[co * P : (co + 1) * P, : 2 * C])
                nc.vector.tensor_copy(wqk[:, co, :], st[:])
                st2 = stage_pool.tile([P, 2 * C], F32, tag="stage", name="st_v")
                nc.sync.dma_start(st2[:, :C], w_attn[co * P : (co + 1) * P, 2 * C :])
                nc.vector.tensor_copy(wv[:, co, :], st2[:, :C])
                # W_proj rows permuted: within f-block co, ytn row-group j
                # holds head 4co + JPERM[j], so those W_proj rows go to
                # row-group j.
                st3 = stage_pool.tile([P, 2 * C], F32, tag="stage", name="st_p")
                for j in range(4):
                    c0 = (4 * co + JPERM[j]) * D
                    nc.sync.dma_start(
                        st3[32 * j : 32 * j + 32, :C], w_proj[c0 : c0 + D, :]
                    )
                nc.vector.tensor_copy(wp[:, co, :], st3[:, :C])

            nc.sync.dma_start(
                battn[:], b_attn[: 2 * C].rearrange("(g p) -> p g", p=P)
            )
            st4 = stage_pool.tile([P, 2 * C], F32, tag="stage", name="st_b")
            nc.sync.dma_start(
                st4[0:1, :C], b_attn[2 * C :].rearrange("(o f) -> o f", o=1)
            )
            nc.vector.tensor_copy(bv_row[:], st4[0:1, :C])

        # b_proj is added on the host after the gather (the projection is
        # the final linear step, so this is exact); only b_v needs a
        # broadcast tile on-chip.
        bv_bcast = const_pool.tile([P, C], F16, tag="bv_bcast")
        ps = mm_psum.tile([P, C], F32, tag="mm", name="bps")
        nc.tensor.matmul(ps[:], ones_r[0:1, :], bv_row[:], start=True, stop=True)
        nc.scalar.copy(bv_bcast[:], ps[:])

        mask_b = masktri.rearrange("p (a f) -> p a f", a=1).to_broadcast(
            [P, HPG, P]
        )

        # ---- per-batch stage emitters -------------------------------
        def emit_transposes(b, xt_b):
            for tb in range(NTBLK):
                xn = xn_pool.tile([P, C], F32, tag="xn", name="xn")
                nc.sync.dma_start(xn[:], x[b, tb * P : (tb + 1) * P, :])
                tp = mm_psum.tile([P, C], F32, tag="mm", name="tps")
                for cb in range(NCBLK):
                    nc.tensor.transpose(
                        tp[:, cb * P : (cb + 1) * P],
                        xn[:, cb * P : (cb + 1) * P],
                        identity[:],
                    )
                nc.vector.tensor_copy(
                    xt_b[:, :, tb * P : (tb + 1) * P],
                    tp.rearrange("p (c t) -> p c t", c=NCBLK),
                )

        def emit_qk(b, xt_b, qt_b, kt_b, groups):
            for g8 in groups:
                ps = mm_psum.tile([P, T], F32, tag="mm", name="qkps")
                for co in range(NCBLK):
                    nc.tensor.matmul(
                        ps[:],
                        wqk[:, co, g8 * P : (g8 + 1) * P],
                        xt_b[:, co, :],
                        start=(co == 0),
                        stop=(co == NCBLK - 1),
                    )
                dst = qt_b[:, g8, :] if g8 < NHG else kt_b[:, g8 - NHG, :]
                nc.vector.tensor_scalar_add(dst, ps[:], battn[:, g8 : g8 + 1])

        def emit_v(b, xt_b, v_b):
            for tb in range(NTBLK):
                ps = mm_psum.tile([P, C], F32, tag="mm", name="vps")
                for co in range(NCBLK):
                    nc.tensor.matmul(
                        ps[:],
                        xt_b[:, co, tb * P : (tb + 1) * P],
                        wv[:, co, :],
                        start=(co == 0),
                        stop=(co == NCBLK - 1),
                    )
                nc.vector.tensor_copy(
                    v_b[:, tb, :, :], ps.rearrange("p (h d) -> p h d", h=H)
                )
                nc.gpsimd.tensor_tensor(
                    v_b[:, tb, :, :],
                    v_b[:, tb, :, :],
                    bv_bcast.rearrange("p (h d) -> p h d", h=H),
                    ALU.add,
                )

        def emit_scores_exp(b, g, qt_b, kt_b):
            """Scores + exp + causal mask for head group g. Returns the 4
            E' tiles (one per k-block i), each [P, HPG, n_i] fp16."""
            e_tiles = []
            for i in range(NTBLK):
                q0 = i * P
                n = T - q0
                sps = s_psum.tile(
                    [P, HPG, T], F32, tag="s", name="sps"
                )
                for hp in range(HPG):
                    nc.tensor.matmul(
                        sps[:, hp, :n],
                        kt_b[32 * hp : 32 * hp + 32, g, q0 : q0 + P],
                        qt_b[32 * hp : 32 * hp + 32, g, q0:T],
                        start=True,
                        stop=True,
                        tile_position=(32 * hp, 0),
                    )
                ep = e_pool.tile([P, HPG, n], F16, tag=f"e{i}", name="ep")
                nc.scalar.activation(ep[:], sps[:, :, :n], AF.Exp, scale=SCALE)
                # causal mask on the diagonal block (local cols [0, 128)):
                # keep iff q - k >= 0, i.e. multiply by the triangle mask.
                nc.gpsimd.tensor_tensor(
                    ep[:, :, 0:P], ep[:, :, 0:P], mask_b, ALU.mult
                )
                e_tiles.append((ep, n, q0))
            return e_tiles

        def emit_yt(b, g, v_b, e_tiles, ytn_b):
            """yT + denominators for head group g, normalized into
            ytn_b[:, g, :].

            Bank A rows: y(h0)@0-31, y(h1)@32-63, den(h2)@64-95,
                         den(h3)@96-127
            Bank B rows: den(h0)@0-31, den(h1)@32-63, y(h2)@64-95,
                         y(h3)@96-127
            so recip(B)[0:64] aligns with A's y half and recip(A)[64:128]
            with B's, and each normalize multiply covers one contiguous
            64-partition half.
            """
            ypsA = y_psum.tile([P, T], F32, tag="y", name="ypsA")
            ypsB = y_psum.tile([P, T], F32, tag="y", name="ypsB")
            for i in range(NTBLK):
                ep, n, q0 = e_tiles[i]
                movs = [ep[:, hp, :n] for hp in range(HPG)]
                vh = [v_b[:, i, HPG * g + hp, :] for hp in range(HPG)]
                specsA = [
                    (0, vh[0], movs[0]),
                    (32, vh[1], movs[1]),
                    (64, ones32, movs[2]),
                    (96, ones32, movs[3]),
                ]
                specsB = [
                    (0, ones32, movs[0]),
                    (32, ones32, movs[1]),
                    (64, vh[2], movs[2]),
                    (96, vh[3], movs[3]),
                ]
                for yps, specs in ((ypsA, specsA), (ypsB, specsB)):
                    for cj, stat, mov in specs:
                        nc.tensor.matmul(
                            yps[cj : cj + 32, q0:T],
                            stat,
                            mov,
                            start=(i == 0),
                            stop=(i == NTBLK - 1),
                            tile_position=(0, cj),
                        )
            recA = rec_pool.tile([P, T], F32, tag="rec", name="recA")
            recB = rec_pool.tile([P, T], F32, tag="rec", name="recB")
            nc.vector.reciprocal_approx_fast(recA[:], ypsA[:])
            nc.vector.reciprocal_approx_fast(recB[:], ypsB[:])
            nc.vector.tensor_tensor(
                ytn_b[0:64, g, :], ypsA[0:64, :], recB[0:64, :], ALU.mult
            )
            nc.vector.tensor_tensor(
                ytn_b[64:128, g, :], ypsB[64:128, :], recA[64:128, :], ALU.mult
            )

        def emit_proj(b, ytn_b, tbs):
            for tb in tbs:
                ps = mm_psum.tile([P, C], F32, tag="mm", name="ops")
                for cb in range(NCBLK):
                    nc.tensor.matmul(
                        ps[:],
                        ytn_b[:, cb, tb * P : (tb + 1) * P],
                        wp[:, cb, :],
                        start=(cb == 0),
                        stop=(cb == NCBLK - 1),
                    )
                ob = o_pool.tile([P, C], F16, tag="obuf", name="ob")
                nc.vector.tensor_copy(ob[:], ps[:])
                nc.sync.dma_start(out[b, tb * P : (tb + 1) * P, :], ob[:])

        # ---- software-pipelined emission ----------------------------
        # Iteration b emits: attention for batch b, interleaved at head-
        # group granularity with transpose/QKV for batch b+1 and the
        # output projection for batch b-1, keeping the PE queue dense
        # across the ScalarE-bound exp phases.
        state = {}  # batch -> (xt, qt, kt, v, ytn)

        def fill_slices(nb):
            """Per-g filler work emitted between attention groups of the
            current batch: stages of batch nb (and proj of nb-2)."""
            if nb >= b_count:
                return [lambda: None] * 4
            xt_b = xt_pool.tile([P, NCBLK, T], F16, tag="xt", name="xt")
            qt_b = qt_pool.tile([P, NHG, T], F16, tag="qt", name="qt")
            kt_b = kt_pool.tile([P, NHG, T], F16, tag="kt", name="kt")
            v_b = v_pool.tile([P, NTBLK, H, D], F16, tag="vnat", name="vb")
            state[nb] = (xt_b, qt_b, kt_b, v_b)
            return [
                lambda: emit_transposes(nb, xt_b),
                lambda: emit_qk(nb, xt_b, qt_b, kt_b, range(0, 4)),
                lambda: emit_qk(nb, xt_b, qt_b, kt_b, range(4, 8)),
                lambda: emit_v(nb, xt_b, v_b),
            ]

        # prologue: stages for batch 0
        slices0 = fill_slices(0)
        for s in slices0:
            s()

        ytn_prev = None
        for b in range(b_count):
            xt_b, qt_b, kt_b, v_b = state.pop(b)
            ytn_b = ytn_pool.tile([P, NHG, T], F16, tag="ytn", name="ytn")
            filler = fill_slices(b + 1)
            for g in range(NHG):
                e_tiles = emit_scores_exp(b, g, qt_b, kt_b)
                emit_yt(b, g, v_b, e_tiles, ytn_b)
                filler[g]()
                if ytn_prev is not None:
                    emit_proj(b - 1, ytn_prev, [g])
            ytn_prev = ytn_b
        emit_proj(b_count - 1, ytn_prev, range(NTBLK))


_NC_CACHE = None


def build_nc(b_count: int = BC, num_devices: int = N_CORES):
    nc = bacc.Bacc(
        "TRN2", target_bir_lowering=False, debug=False, num_devices=num_devices
    )
    x = nc.dram_tensor("x", [b_count, T, C], F32, kind="ExternalInput").ap()
    w_attn = nc.dram_tensor("w_attn", [C, 3 * C], F32, kind="ExternalInput").ap()
    b_attn = nc.dram_tensor("b_attn", [3 * C], F32, kind="ExternalInput").ap()
    w_proj = nc.dram_tensor("w_proj", [C, C], F32, kind="ExternalInput").ap()
    b_proj = nc.dram_tensor("b_proj", [C], F32, kind="ExternalInput").ap()
    out = nc.dram_tensor("out", [b_count, T, C], F16, kind="ExternalOutput").ap()
    with tile.TileContext(nc) as tc:
        _build_kernel(tc, out, x, w_attn, b_attn, w_proj, b_proj, b_count)
    nc.compile()
    return nc


def _get_nc():
    global _NC_CACHE
    if _NC_CACHE is None:
        _NC_CACHE = build_nc(BC, N_CORES)
    return _NC_CACHE


def kernel(x, W_attn, b_attn, W_proj, b_proj):
    x = np.ascontiguousarray(np.asarray(x, dtype=np.float32))
    W_attn = np.ascontiguousarray(np.asarray(W_attn, dtype=np.float32))
    b_attn = np.ascontiguousarray(np.asarray(b_attn, dtype=np.float32))
    W_proj = np.ascontiguousarray(np.asarray(W_proj, dtype=np.float32))
    b_proj = np.ascontiguousarray(np.asarray(b_proj, dtype=np.float32))

    nc = _get_nc()
    in_maps = [
        {
            "x": x[c * BC : (c + 1) * BC],
            "w_attn": W_attn,
            "b_attn": b_attn,
            "w_proj": W_proj,
            "b_proj": b_proj,
        }
        for c in range(N_CORES)
    ]
    res = run_bass_kernel_spmd(nc, in_maps, core_ids=list(range(N_CORES)))
    full = np.concatenate(
        [res.results[c]["out"].astype(np.float32) for c in range(N_CORES)], axis=0
    )
    return full + b_proj[None, None, :]


# revision 28
# speedup vs baseline: 1.9534x; 1.1559x over previous
"""Causal self-attention (B=128, T=512, C=512, H=16) on 8 Trainium2 NeuronCores.

Sharding: data-parallel over batch — each core computes 16 of the 128
batch elements end-to-end; weights are replicated. No collectives.

Per-core kernel (Bass/Tile; matmul operands fp16, fp32 accumulation),
restructured from the v1 kernel around three findings from its trace:
  1. HAM throttling: the PE ran at K=4/8 (1.2 GHz) for 82% of the run
     because per-batch serialization (normalize tail -> proj -> next
     batch through one shared PSUM bank) left 200ns-5.4us PE gaps.
     The emission schedule now interleaves, at head-group granularity,
     batch b's attention with batch b+1's transpose/QKV and batch b-1's
     output projection so the PE stream stays dense.
  2. ScalarE exp paid a 352-cycle fixed cost on each of 64 small
     ACTIVATEs per batch. Scores for the 4 heads of a group now land in
     4 PSUM banks of one 4-bank tile and are exp'd by ONE ACTIVATE over
     a bank-strided AP (16 per batch).
  3. The normalize chain (1-partition reciprocal + gpsimd
     partition_broadcast per head, 440us combined) is gone: each y-PSUM
     bank col-tiles y of two heads with the ones-matmul denominators of
     the OTHER two heads placed at complementary partition rows, so a
     full-bank reciprocal_approx_fast of bank A lands 1/den exactly on
     the partitions where bank B holds y (and vice versa); three DVE
     multiplies then write normalized yT directly into the packed
     stationary layout for the projection (W_proj rows permuted at load
     to match).

Causal masking multiplies the diagonal 128-block of each exp'd tile by
a constant lower-triangle mask on GpSimd (one op per (group, k-block)).
"""

import math
import sys

if "/opt/trn_rl_repo" not in sys.path:
    sys.path.insert(0, "/opt/trn_rl_repo")

import numpy as np

import concourse.tile as tile
from concourse import bacc, mybir
from concourse.bass_utils import run_bass_kernel_spmd
from concourse.masks import make_identity

F32 = mybir.dt.float32
F16 = mybir.dt.float16  # fp16: full matmul rate + FWL, 11-bit mantissa
AF = mybir.ActivationFunctionType
ALU = mybir.AluOpType

B, T, C, H, D = 128, 512, 512, 16, 32
P = 128
N_CORES = 8
BC = B // N_CORES       # 16 batches per core
NCBLK = C // P          # 4
NTBLK = T // P          # 4
HPG = 4                 # heads per 128-row f-block
NHG = H // HPG          # 4 head groups
SCALE = 1.0 / math.sqrt(D)
# ytn row-group j of f-block g holds head 4g + JPERM[j] (see y-bank layout)
JPERM = [0, 1, 2, 3]


def _build_kernel(tc, out, x, w_attn, b_attn, w_proj, b_proj, b_count=BC):
    nc = tc.nc
    with (
        tc.tile_pool(name="const", bufs=1) as const_pool,
        tc.tile_pool(name="xn", bufs=2) as xn_pool,
        tc.tile_pool(name="xt", bufs=2) as xt_pool,
        tc.tile_pool(name="qt", bufs=2) as qt_pool,
        tc.tile_pool(name="kt", bufs=2) as kt_pool,
        tc.tile_pool(name="vnat", bufs=2) as v_pool,
        tc.tile_pool(name="eprime", bufs=2) as e_pool,
        tc.tile_pool(name="rec", bufs=4) as rec_pool,
        tc.tile_pool(name="ytn", bufs=2) as ytn_pool,
        tc.tile_pool(name="obuf", bufs=3) as o_pool,
        tc.tile_pool(name="s_psum", bufs=1, space="PSUM") as s_psum,
        tc.tile_pool(name="y_psum", bufs=2, space="PSUM") as y_psum,
        tc.tile_pool(name="mm_psum", bufs=2, space="PSUM") as mm_psum,
    ):
        identity = const_pool.tile([P, P], F32, tag="identity")
        make_identity(nc, identity)
        ones32 = const_pool.tile([P, D], F16, tag="ones32")
        nc.gpsimd.memset(ones32[:], 1.0)
        # lower-triangle keep mask for the causal diagonal block:
        # masktri[p, f] = 1.0 iff f >= p
        masktri = const_pool.tile([P, P], F16, tag="masktri")
        nc.gpsimd.memset(masktri[:], 1.0)
        nc.gpsimd.affine_select(
            out=masktri[:],
            in_=masktri[:],
            compare_op=ALU.is_ge,
            fill=0.0,
            base=0,
            channel_multiplier=-1,
            pattern=[[1, P]],
        )
        onesf = const_pool.tile([P, P], F32, tag="onesf")
        nc.gpsimd.memset(onesf[:], 1.0)
        ones_r = const_pool.tile([P, P], F16, tag="ones_r")
        nc.vector.tensor_copy(ones_r[:], onesf[:])

        wqk = const_pool.tile([P, NCBLK, 2 * C], F16, tag="wqk")
        wv = const_pool.tile([P, NCBLK, C], F16, tag="wv")
        wp = const_pool.tile([P, NCBLK, C], F16, tag="wp")
        battn = const_pool.tile([P, 8], F32, tag="battn")
        bv_row = const_pool.tile([1, C], F16, tag="bv_row")
        with tc.tile_pool(name="stage", bufs=2) as stage_pool:
            for co in range(NCBLK):
                st = stage_pool.tile([P, 2 * C], F32, tag="stage", name="st_qk")
                nc.sync.dma_start(st[:], w_attn[co * P : (co + 1) * P, : 2 * C])
                nc.vector.tensor_copy(wqk[:, co, :], st[:])
                st2 = stage_pool.tile([P, 2 * C], F32, tag="stage", name="st_v")
                nc.sync.dma_start(st2[:, :C], w_attn[co * P : (co + 1) * P, 2 * C :])
                nc.vector.tensor_copy(wv[:, co, :], st2[:, :C])
                # W_proj rows permuted: within f-block co, ytn row-group j
                # holds head 4co + JPERM[j], so those W_proj rows go to
                # row-group j.
                st3 = stage_pool.tile([P, 2 * C], F32, tag="stage", name="st_p")
                for j in range(4):
                    c0 = (4 * co + JPERM[j]) * D
                    nc.sync.dma_start(
                        st3[32 * j : 32 * j + 32, :C], w_proj[c0 : c0 + D, :]
                    )
                nc.vector.tensor_copy(wp[:, co, :], st3[:, :C])

            nc.sync.dma_start(
                battn[:], b_attn[: 2 * C].rearrange("(g p) -> p g", p=P)
            )
            st4 = stage_pool.tile([P, 2 * C], F32, tag="stage", name="st_b")
            nc.sync.dma_start(
                st4[0:1, :C], b_attn[2 * C :].rearrange("(o f) -> o f", o=1)
            )
            nc.vector.tensor_copy(bv_row[:], st4[0:1, :C])

        # b_proj is added on the host after the gather (the projection is
        # the final linear step, so this is exact); b_v is added via a
        # K=1 rank-1 matmul folded into the V accumulation.

        mask_b = masktri.rearrange("p (a f) -> p a f", a=1).to_broadcast(
            [P, HPG, P]
        )

        # ---- per-batch stage emitters -------------------------------
        def emit_transposes(b, xt_b, tbs):
            for tb in tbs:
                xn = xn_pool.tile([P, C], F32, tag="xn", name="xn")
                nc.sync.dma_start(xn[:], x[b, tb * P : (tb + 1) * P, :])
                tp = mm_psum.tile([P, C], F32, tag="mm", name="tps")
                for cb in range(NCBLK):
                    nc.tensor.transpose(
                        tp[:, cb * P : (cb + 1) * P],
                        xn[:, cb * P : (cb + 1) * P],
                        identity[:],
                    )
                nc.vector.tensor_copy(
                    xt_b[:, :, tb * P : (tb + 1) * P],
                    tp.rearrange("p (c t) -> p c t", c=NCBLK),
                )

        def emit_qk(b, xt_b, qt_b, kt_b, groups):
            for g8 in groups:
                ps = mm_psum.tile([P, T], F32, tag="mm", name="qkps")
                for co in range(NCBLK):
                    nc.tensor.matmul(
                        ps[:],
                        wqk[:, co, g8 * P : (g8 + 1) * P],
                        xt_b[:, co, :],
                        start=(co == 0),
                        stop=(co == NCBLK - 1),
                    )
                dst = qt_b[:, g8, :] if g8 < NHG else kt_b[:, g8 - NHG, :]
                nc.vector.tensor_scalar_add(dst, ps[:], battn[:, g8 : g8 + 1])

        def emit_v(b, xt_b, v_b, tbs):
            for tb in tbs:
                ps = mm_psum.tile([P, C], F32, tag="mm", name="vps")
                for co in range(NCBLK):
                    nc.tensor.matmul(
                        ps[:],
                        xt_b[:, co, tb * P : (tb + 1) * P],
                        wv[:, co, :],
                        start=(co == 0),
                        stop=False,
                    )
                # + bias: K=1 rank-1 update ones^T (x) bv_row
                nc.tensor.matmul(
                    ps[:], ones_r[0:1, :], bv_row[:], start=False, stop=True
                )
                nc.vector.tensor_copy(
                    v_b[:, tb, :, :], ps.rearrange("p (h d) -> p h d", h=H)
                )

        def emit_scores_exp_i(b, g, i, qt_b, kt_b):
            """Scores + exp + causal mask for head group g, k-block i.
            Returns the E' tile [P, HPG, n_i] fp16."""
            q0 = i * P
            n = T - q0
            sps = s_psum.tile([P, HPG, T], F32, tag="s", name="sps")
            for hp in range(HPG):
                nc.tensor.matmul(
                    sps[:, hp, :n],
                    kt_b[32 * hp : 32 * hp + 32, g, q0 : q0 + P],
                    qt_b[32 * hp : 32 * hp + 32, g, q0:T],
                    start=True,
                    stop=True,
                    tile_position=(32 * hp, 0),
                )
            ep = e_pool.tile([P, HPG, n], F16, tag=f"e{i}", name="ep")
            nc.scalar.activation(ep[:], sps[:, :, :n], AF.Exp, scale=SCALE)
            # causal mask on the diagonal block (local cols [0, 128)):
            # keep iff q - k >= 0, i.e. multiply by the triangle mask.
            nc.gpsimd.tensor_tensor(
                ep[:, :, 0:P], ep[:, :, 0:P], mask_b, ALU.mult
            )
            return (ep, n, q0)

        def emit_yt_i(g, v_b, e_tile, ypsA, ypsB, i):
            """One k-block's worth of yT + denominator accumulation."""
            ep, n, q0 = e_tile
            movs = [ep[:, hp, :n] for hp in range(HPG)]
            vh = [v_b[:, i, HPG * g + hp, :] for hp in range(HPG)]
            specsA = [
                (0, vh[0], movs[0]),
                (32, vh[1], movs[1]),
                (64, ones32, movs[2]),
                (96, ones32, movs[3]),
            ]
            specsB = [
                (0, ones32, movs[0]),
                (32, ones32, movs[1]),
                (64, vh[2], movs[2]),
                (96, vh[3], movs[3]),
            ]
            for yps, specs in ((ypsA, specsA), (ypsB, specsB)):
                for cj, stat, mov in specs:
                    nc.tensor.matmul(
                        yps[cj : cj + 32, q0:T],
                        stat,
                        mov,
                        start=(i == 0),
                        stop=(i == NTBLK - 1),
                        tile_position=(0, cj),
                    )

        def emit_normalize(g, ypsA, ypsB, ytn_b):
            """Normalize y into ytn_b[:, g, :].

            Bank A rows: y(h0)@0-31, y(h1)@32-63, den(h2)@64-95,
                         den(h3)@96-127
            Bank B rows: den(h0)@0-31, den(h1)@32-63, y(h2)@64-95,
                         y(h3)@96-127
            so recip(B)[0:64] aligns with A's y half and recip(A)[64:128]
            with B's, and each normalize multiply covers one contiguous
            64-partition half.
            """
            recA = rec_pool.tile([P, T], F32, tag="rec", name="recA")
            recB = rec_pool.tile([P, T], F32, tag="rec", name="recB")
            nc.vector.reciprocal_approx_fast(recA[:], ypsA[:])
            nc.vector.reciprocal_approx_fast(recB[:], ypsB[:])
            nc.vector.tensor_tensor(
                ytn_b[0:64, g, :], ypsA[0:64, :], recB[0:64, :], ALU.mult
            )
            nc.vector.tensor_tensor(
                ytn_b[64:128, g, :], ypsB[64:128, :], recA[64:128, :], ALU.mult
            )

        def emit_proj(b, ytn_b, tbs):
            for tb in tbs:
                ps = mm_psum.tile([P, C], F32, tag="mm", name="ops")
                for cb in range(NCBLK):
                    nc.tensor.matmul(
                        ps[:],
                        ytn_b[:, cb, tb * P : (tb + 1) * P],
                        wp[:, cb, :],
                        start=(cb == 0),
                        stop=(cb == NCBLK - 1),
                    )
                ob = o_pool.tile([P, C], F16, tag="obuf", name="ob")
                nc.vector.tensor_copy(ob[:], ps[:])
                nc.sync.dma_start(out[b, tb * P : (tb + 1) * P, :], ob[:])

        # ---- software-pipelined emission ----------------------------
        # Iteration b emits attention for batch b, interleaved at
        # (head-group, k-block) granularity with transpose/QKV filler
        # chunks for batch b+1 and the output projection for batch b-1,
        # so the PE queue always holds ready work across the
        # ScalarE-bound exp phases (s_psum WAR stalls).  Within a group,
        # yT of k-block i is emitted after scores of k-block i+1 so it
        # covers ACT(i+1)'s latency.
        state = {}  # batch -> (xt, qt, kt, v)

        def fill_chunks(nb):
            """16 filler chunks (one per (g, i) slot) building batch nb's
            transposes and QKV projections."""
            if nb >= b_count:
                return [lambda: None] * 16
            xt_b = xt_pool.tile([P, NCBLK, T], F16, tag="xt", name="xt")
            qt_b = qt_pool.tile([P, NHG, T], F16, tag="qt", name="qt")
            kt_b = kt_pool.tile([P, NHG, T], F16, tag="kt", name="kt")
            v_b = v_pool.tile([P, NTBLK, H, D], F16, tag="vnat", name="vb")
            state[nb] = (xt_b, qt_b, kt_b, v_b)
            chunks = []
            for tb in range(NTBLK):
                chunks.append(lambda tb=tb: emit_transposes(nb, xt_b, [tb]))
            for g8 in range(2 * NHG):
                chunks.append(
                    lambda g8=g8: emit_qk(nb, xt_b, qt_b, kt_b, [g8])
                )
            for tb in range(NTBLK):
                chunks.append(lambda tb=tb: emit_v(nb, xt_b, v_b, [tb]))
            return chunks

        # prologue: stages for batch 0
        for ch in fill_chunks(0):
            ch()

        ytn_prev = None
        for b in range(b_count):
            xt_b, qt_b, kt_b, v_b = state.pop(b)
            ytn_b = ytn_pool.tile([P, NHG, T], F16, tag="ytn", name="ytn")
            chunks = fill_chunks(b + 1)
            for g in range(NHG):
                ypsA = y_psum.tile([P, T], F32, tag="y", name="ypsA")
                ypsB = y_psum.tile([P, T], F32, tag="y", name="ypsB")
                e_prev = None
                for i in range(NTBLK):
                    e_cur = emit_scores_exp_i(b, g, i, qt_b, kt_b)
                    if e_prev is not None:
                        chunks[4 * g + i]()
                        emit_yt_i(g, v_b, e_prev, ypsA, ypsB, i - 1)
                    e_prev = e_cur
                chunks[4 * g + 0]()
                emit_yt_i(g, v_b, e_prev, ypsA, ypsB, NTBLK - 1)
                emit_normalize(g, ypsA, ypsB, ytn_b)
                if ytn_prev is not None:
                    emit_proj(b - 1, ytn_prev, [g])
            ytn_prev = ytn_b
        emit_proj(b_count - 1, ytn_prev, range(NTBLK))


_NC_CACHE = None


def build_nc(b_count: int = BC, num_devices: int = N_CORES):
    nc = bacc.Bacc(
        "TRN2", target_bir_lowering=False, debug=False, num_devices=num_devices
    )
    x = nc.dram_tensor("x", [b_count, T, C], F32, kind="ExternalInput").ap()
    w_attn = nc.dram_tensor("w_attn", [C, 3 * C], F32, kind="ExternalInput").ap()
    b_attn = nc.dram_tensor("b_attn", [3 * C], F32, kind="ExternalInput").ap()
    w_proj = nc.dram_tensor("w_proj", [C, C], F32, kind="ExternalInput").ap()
    b_proj = nc.dram_tensor("b_proj", [C], F32, kind="ExternalInput").ap()
    out = nc.dram_tensor("out", [b_count, T, C], F16, kind="ExternalOutput").ap()
    with tile.TileContext(nc) as tc:
        _build_kernel(tc, out, x, w_attn, b_attn, w_proj, b_proj, b_count)
    nc.compile()
    return nc


def _get_nc():
    global _NC_CACHE
    if _NC_CACHE is None:
        _NC_CACHE = build_nc(BC, N_CORES)
    return _NC_CACHE


def kernel(x, W_attn, b_attn, W_proj, b_proj):
    x = np.ascontiguousarray(np.asarray(x, dtype=np.float32))
    W_attn = np.ascontiguousarray(np.asarray(W_attn, dtype=np.float32))
    b_attn = np.ascontiguousarray(np.asarray(b_attn, dtype=np.float32))
    W_proj = np.ascontiguousarray(np.asarray(W_proj, dtype=np.float32))
    b_proj = np.ascontiguousarray(np.asarray(b_proj, dtype=np.float32))

    nc = _get_nc()
    in_maps = [
        {
            "x": x[c * BC : (c + 1) * BC],
            "w_attn": W_attn,
            "b_attn": b_attn,
            "w_proj": W_proj,
            "b_proj": b_proj,
        }
        for c in range(N_CORES)
    ]
    res = run_bass_kernel_spmd(nc, in_maps, core_ids=list(range(N_CORES)))
    full = np.concatenate(
        [res.results[c]["out"].astype(np.float32) for c in range(N_CORES)], axis=0
    )
    return full + b_proj[None, None, :]
